# revision 20
# baseline (speedup 1.0000x reference)
"""GAT (3-layer graph attention + final linear) Trainium2 Bass kernel.

Problem: B=4 graphs, N=2048 atoms, D=128, H=256.
  per layer: h = relu(x @ W.T + b); e_ij = leaky_relu(f1_i + f2_j, 0.01)
  masked by adj; att = softmax_j(e); x = x + att @ h.
  final: relu(x @ Wt.T + bt).

Sharding: 8 cores; core c -> (graph b=c//2, row-half s=c%2 of the NxN
attention). Each core computes full h for its graph (cheap), attention
only for its 1024 rows. Between layers the updated x rows are exchanged
within (2b, 2b+1) pairs via AllGather.

Key tricks:
  - additive mask maskT[j,i] = 9e15*adj[i,j] - 9e15 precomputed once
    (transposed via PE, stored bf16), kept in SBUF; each layer a PE
    identity-matmul writes it into PSUM (start=True) and one K=2 outer
    product [f2;1]x[1;f1] accumulates the logits on top, so masking
    costs no separate NxN pass (and no DVE time).
  - softmax without row-max: logits bounded (~36) so exp is safe in f32;
    validated vs reference (rel err 3.6e-7).
  - exp(leaky(z)) = max(exp(z), exp(0.01*z)): two ACT Exp passes (same
    LUT set, no table reloads) + one DVE max.
  - attention aggregated transposed (attoutT = hnat[j]^T @ p, one
    stationary load per j-tile, 512-wide bf16 streams); row sums via a
    ones-column matmul; normalization happens after transposing back,
    on [NS,D] instead of an NxN pass.

Hardware wait-slot discipline (walrus limits: DMA instr = 1 sem wait,
engine instr = 2): every DMA is a first-write to its slot or its slot
was last touched by a single full-coverage engine write (gpsimd memset);
small weights are "laundered" through DVE copies so no matmul depends on
two DMA queues.
"""

import numpy as np

import concourse.bass as bass
import concourse.mybir as mybir
import concourse.tile as tile
from concourse import masks
from concourse.bass_utils import run_bass_kernel_spmd

P = 128
F32 = mybir.dt.float32
BF16 = mybir.dt.bfloat16
I32 = mybir.dt.int32
AF = mybir.ActivationFunctionType
OP = mybir.AluOpType

NEGC = 9e15


def _legalize_waits(nc, dma_limit=1, engine_limit=1):
    """Walrus can encode only 1 sem wait on a DMA instruction and ~2 on an
    engine instruction. Move excess waits onto standalone EventSemaphore
    instructions (1 wait each) inserted just before the offender on the
    same engine."""
    counter = [0]

    def split(ins):
        si = ins.sync_info
        if si is None:
            return None
        limit = dma_limit if type(ins).__name__.startswith("InstDMA") \
            else engine_limit
        waits = list(si.on_wait)
        if len(waits) <= limit:
            return None
        keep = waits[-limit:] if limit > 0 else []
        extra = waits[:-limit] if limit > 0 else waits
        evs = []
        for w in extra:
            counter[0] += 1
            evs.append(mybir.InstEventSemaphore(
                name=f"evsplit{counter[0]}", engine=ins.engine,
                sync_info=mybir.SyncInfo(on_wait=[w], on_update=[])))
        ins.sync_info = mybir.SyncInfo(on_wait=keep,
                                       on_update=list(si.on_update))
        return evs

    for f in nc.m.functions:
        for blk in f.blocks:
            new_list = []
            changed = False
            for ins in blk.instructions:
                evs = split(ins)
                if evs:
                    new_list.extend(evs)
                    changed = True
                new_list.append(ins)
            if changed:
                blk.instructions = new_list


def build_gat_nc(N, NS, D, H, num_cores, pair_groups, nlayers=3,
                 legalize=True):
    assert D == P and NS % 512 == 0 and N % 512 == 0
    nj = N // P        # j tiles
    nit = NS // P      # i tiles in shard
    nch = NS // 512    # 512-chunks in shard
    nchN = N // 512
    nH = H // P

    nc = bass.Bass("TRN2", target_bir_lowering=False, debug=False,
                   num_devices=num_cores)

    # ---- I/O ----
    xT_in = nc.dram_tensor("xT", [P, N], F32, kind="ExternalInput")
    xTs_in = nc.dram_tensor("xTs", [P, NS], F32, kind="ExternalInput")
    xs_in = nc.dram_tensor("xs", [NS, D], F32, kind="ExternalInput")
    adj_in = nc.dram_tensor("adj_s", [NS, N], I32, kind="ExternalInput")
    WT_in = [nc.dram_tensor(f"WT{l}", [D, D], F32, kind="ExternalInput")
             for l in range(nlayers)]
    bv_in = [nc.dram_tensor(f"bv{l}", [D, 1], F32, kind="ExternalInput")
             for l in range(nlayers)]
    av_in = [nc.dram_tensor(f"av{l}", [D, 2], F32, kind="ExternalInput")
             for l in range(nlayers)]
    WtT_in = nc.dram_tensor("WtT", [D, H], F32, kind="ExternalInput")
    btp_in = nc.dram_tensor("btp", [P, nH], F32, kind="ExternalInput")
    out_ext = nc.dram_tensor("out_s", [NS, H], F32, kind="ExternalOutput")

    # DRAM bounce buffers for the pair AllGather of xT shards
    ag_in = [nc.dram_tensor(f"ag_in{l}", [P, NS], F32)
             for l in range(nlayers - 1)]
    ag_out = [nc.dram_tensor(f"ag_out{l}", [2 * P, NS], F32)
              for l in range(nlayers - 1)]

    with tile.TileContext(nc) as tc:
        import contextlib
        ctx = contextlib.ExitStack()
        with ctx:
            persist = ctx.enter_context(tc.tile_pool(name="persist", bufs=1))
            rawp = ctx.enter_context(tc.tile_pool(name="rawp", bufs=2))
            convp = ctx.enter_context(tc.tile_pool(name="convp", bufs=4))
            qp = ctx.enter_context(tc.tile_pool(name="qp", bufs=3))
            xsp = ctx.enter_context(tc.tile_pool(name="xsp", bufs=2))
            xtp = ctx.enter_context(tc.tile_pool(name="xtp", bufs=2))
            smallp = ctx.enter_context(tc.tile_pool(name="smallp", bufs=4))
            ocp = ctx.enter_context(tc.tile_pool(name="ocp", bufs=2))
            pe_pool = ctx.enter_context(
                tc.tile_pool(name="pe_pool", bufs=2, space="PSUM"))
            attp = ctx.enter_context(
                tc.tile_pool(name="attp", bufs=1, space="PSUM"))
            spp = ctx.enter_context(
                tc.tile_pool(name="spp", bufs=1, space="PSUM"))

            ident = persist.tile([P, P], F32)
            masks.make_identity(nc, ident[:])
            identb = persist.tile([P, P], BF16)
            masks.make_identity(nc, identb[:])

            # ---- persistent state ----
            maskT = [persist.tile([P, NS], BF16, name=f"maskT{j}",
                                  tag=f"maskT{j}") for j in range(nj)]
            hT = persist.tile([P, N], F32)
            hsT = persist.tile([P, NS], F32)
            hnat = [persist.tile([P, D], BF16, name=f"hnat{j}",
                                 tag=f"hnat{j}") for j in range(nj)]
            onesrow = persist.tile([1, N], F32)
            nc.vector.memset(onesrow[:], 1.0)
            onescol = persist.tile([P, 1], BF16)
            nc.vector.memset(onescol[:], 1.0)
            f1row = persist.tile([1, NS], F32)
            frowA = persist.tile([2, N], F32)   # [f2 ; ones]
            frowB = persist.tile([2, NS], F32)  # [ones ; f1]
            nc.sync.dma_start(frowA[1:2, :], onesrow[:])
            nc.sync.dma_start(frowB[0:1, :], onesrow[:, 0:NS])

            # raw DMA'd weights + DVE-laundered copies (so matmuls never
            # depend on two DMA queues)
            WT_d = [persist.tile([D, D], F32, name=f"WTd{l}", tag=f"WTd{l}")
                    for l in range(nlayers)]
            bv_d = [persist.tile([D, 1], F32, name=f"bvd{l}", tag=f"bvd{l}")
                    for l in range(nlayers)]
            av_d = [persist.tile([D, 2], F32, name=f"avd{l}", tag=f"avd{l}")
                    for l in range(nlayers)]
            WtT_d = persist.tile([D, H], F32)
            btp_d = persist.tile([P, nH], F32)
            WT = [persist.tile([D, D], F32, name=f"WTl{l}", tag=f"WTl{l}")
                  for l in range(nlayers)]
            bv = [persist.tile([D, 1], F32, name=f"bvl{l}", tag=f"bvl{l}")
                  for l in range(nlayers)]
            av = [persist.tile([D, 2], F32, name=f"avl{l}", tag=f"avl{l}")
                  for l in range(nlayers)]
            WtTt = persist.tile([D, H], F32)
            btpt = persist.tile([P, nH], F32)
            for l in range(nlayers):
                nc.sync.dma_start(WT_d[l][:], WT_in[l].ap())
                nc.sync.dma_start(bv_d[l][:], bv_in[l].ap())
                nc.sync.dma_start(av_d[l][:], av_in[l].ap())
                nc.vector.tensor_copy(WT[l][:], WT_d[l][:])
                nc.vector.tensor_copy(bv[l][:], bv_d[l][:])
                nc.vector.tensor_copy(av[l][:], av_d[l][:])
            nc.sync.dma_start(WtT_d[:], WtT_in.ap())
            nc.sync.dma_start(btp_d[:], btp_in.ap())
            nc.vector.tensor_copy(WtTt[:], WtT_d[:])
            nc.vector.tensor_copy(btpt[:], btp_d[:])

            # ---- preprocessing: maskT[j][:, i] = 9e15*adj[i, j] - 9e15 ----
            # raw slots are "closed" with a full-coverage gpsimd memset so
            # the next DMA into the slot has exactly one wait.
            for itg in range(nit // 4):
                convs = []
                for q in range(4):
                    it = itg * 4 + q
                    raw = rawp.tile([P, N], I32, name=f"raw{it}", tag="raw")
                    nc.sync.dma_start(raw[:],
                                      adj_in.ap()[it * P:(it + 1) * P, :])
                    conv = convp.tile([P, N], BF16, name=f"conv{it}",
                                      tag="conv")
                    nc.vector.tensor_scalar(conv[:], raw[:], NEGC, -NEGC,
                                            OP.mult, OP.add)
                    nc.gpsimd.memset(raw[:], 0)
                    convs.append(conv)
                for j in range(nj):
                    pst = pe_pool.tile([P, 512], BF16, name=f"tp{itg}_{j}",
                                       tag="pe")
                    for q in range(4):
                        nc.tensor.transpose(pst[:, q * P:(q + 1) * P],
                                            convs[q][:, j * P:(j + 1) * P],
                                            identb[:])
                    nc.vector.tensor_copy(
                        maskT[j][:, itg * 512:(itg + 1) * 512], pst[:])

            # ---- initial x state ----
            xT = xtp.tile([P, N], F32, name="xT0", tag="xT", bufs=3)
            nc.sync.dma_start(xT[:], xT_in.ap())
            xTs = xtp.tile([P, NS], F32, name="xTs0", tag="xTs")
            nc.sync.dma_start(xTs[:], xTs_in.ap())
            xs = []
            for k in range(nit):
                t = xsp.tile([P, D], F32, name=f"xs0_{k}", tag=f"xs{k}")
                nc.sync.dma_start(t[:], xs_in.ap()[k * P:(k + 1) * P, :])
                xs.append(t)

            for l in range(nlayers):
                last = l == nlayers - 1
                # h full (transposed): hT = relu(WT.T @ xT + b)
                for ch in range(nchN):
                    ps = pe_pool.tile([P, 512], F32, name=f"hps{l}_{ch}",
                                      tag="pe")
                    nc.tensor.matmul(ps[:], WT[l][:],
                                     xT[:, ch * 512:(ch + 1) * 512],
                                     start=True, stop=True)
                    nc.vector.tensor_scalar(hT[:, ch * 512:(ch + 1) * 512],
                                            ps[:], bv[l][:], 0.0,
                                            OP.add, OP.max)
                # h shard (transposed)
                for ch in range(nch):
                    ps = pe_pool.tile([P, 512], F32, name=f"hsps{l}_{ch}",
                                      tag="pe")
                    nc.tensor.matmul(ps[:], WT[l][:],
                                     xTs[:, ch * 512:(ch + 1) * 512],
                                     start=True, stop=True)
                    nc.vector.tensor_scalar(hsT[:, ch * 512:(ch + 1) * 512],
                                            ps[:], bv[l][:], 0.0,
                                            OP.add, OP.max)
                # f2 over all atoms / f1 over shard -> partition-0 rows
                for ch in range(nchN):
                    ps = pe_pool.tile([1, 512], F32, name=f"f2ps{l}_{ch}",
                                      tag="pe")
                    nc.tensor.matmul(ps[:], av[l][:, 1:2],
                                     hT[:, ch * 512:(ch + 1) * 512],
                                     start=True, stop=True)
                    nc.vector.tensor_copy(
                        frowA[0:1, ch * 512:(ch + 1) * 512], ps[0:1, :])
                for ch in range(nch):
                    ps = pe_pool.tile([1, 512], F32, name=f"f1ps{l}_{ch}",
                                      tag="pe")
                    nc.tensor.matmul(ps[:], av[l][:, 0:1],
                                     hsT[:, ch * 512:(ch + 1) * 512],
                                     start=True, stop=True)
                    nc.vector.tensor_copy(
                        f1row[0:1, ch * 512:(ch + 1) * 512], ps[0:1, :])
                nc.sync.dma_start(frowB[1:2, :], f1row[:])

                # hext: natural-layout h tiles (transpose hT) + ones column
                for g in range(nj // 4):
                    pst = pe_pool.tile([P, 512], F32, name=f"htp{l}_{g}",
                                       tag="pe")
                    for q in range(4):
                        j = g * 4 + q
                        nc.tensor.transpose(pst[:, q * P:(q + 1) * P],
                                            hT[:, j * P:(j + 1) * P],
                                            ident[:])
                    for q in range(4):
                        j = g * 4 + q
                        nc.vector.tensor_copy(hnat[j][:],
                                              pst[:, q * P:(q + 1) * P])

                # ---- attention + aggregation (transposed accum) ----
                psAT = attp.tile([P, NS], F32, name=f"psAT{l}", tag="att")
                psS = spp.tile([1, NS], F32, name=f"psS{l}", tag="s")
                for j in range(nj):
                    pe = pe_pool.tile([P, NS], F32, name=f"pe{l}_{j}",
                                      tag="pe")
                    for ch in range(nch):
                        sl = slice(ch * 512, (ch + 1) * 512)
                        # mask preload via PE identity-matmul (bf16)
                        nc.tensor.matmul(pe[:, sl], identb[:],
                                         maskT[j][:, sl],
                                         start=True, stop=False)
                        # += f2_j x ones + ones x f1_i  (K=2)
                        nc.tensor.matmul(pe[:, sl],
                                         frowA[0:2, j * P:(j + 1) * P],
                                         frowB[0:2, sl],
                                         start=False, stop=True)
                    # exp(leaky(z)) = max(exp(z), exp(0.01 z)), in bf16
                    q1 = qp.tile([P, NS], BF16, name=f"q1_{l}_{j}", tag="q1")
                    nc.scalar.activation(q1[:], pe[:], AF.Exp)
                    q2 = qp.tile([P, NS], BF16, name=f"q2_{l}_{j}", tag="q2")
                    nc.scalar.activation(q2[:], pe[:], AF.Exp, scale=0.01)
                    p = q1
                    nc.vector.tensor_tensor(p[:], q1[:], q2[:], OP.max)
                    for ch in range(nch):
                        sl = slice(ch * 512, (ch + 1) * 512)
                        nc.tensor.matmul(psAT[:, sl], hnat[j][:], p[:, sl],
                                         start=(j == 0), stop=(j == nj - 1))
                        nc.tensor.matmul(psS[0:1, sl], onescol[:], p[:, sl],
                                         start=(j == 0), stop=(j == nj - 1))

                # normalize + residual -> new xs tiles
                aT = qp.tile([P, NS], F32, name=f"aT{l}", tag="aT")
                nc.vector.tensor_copy(aT[:], psAT[:])
                s_row = smallp.tile([1, NS], F32, name=f"srow{l}",
                                    tag="srow")
                nc.vector.tensor_copy(s_row[:], psS[:])
                # s row -> per-partition column via PE transpose
                stp = pe_pool.tile([P, nit], F32, name=f"stp{l}", tag="pe")
                for it in range(nit):
                    nc.tensor.transpose(stp[:, it:it + 1],
                                        s_row[0:1, it * P:(it + 1) * P],
                                        ident[0:1, 0:1])
                rss = []
                for it in range(nit):
                    rs = smallp.tile([P, 1], F32, name=f"rs{l}_{it}",
                                     tag="rs", bufs=8)
                    nc.vector.reciprocal(rs[:], stp[:, it:it + 1])
                    rss.append(rs)
                xs_new = []
                for g2 in range(nit // 4):
                    atp = pe_pool.tile([P, 512], F32, name=f"atp{l}_{g2}",
                                       tag="pe")
                    for q in range(4):
                        it = g2 * 4 + q
                        nc.tensor.transpose(atp[:, q * P:(q + 1) * P],
                                            aT[:, it * P:(it + 1) * P],
                                            ident[:])
                    for q in range(4):
                        it = g2 * 4 + q
                        tmp = smallp.tile([P, D], F32, name=f"tmp{l}_{it}",
                                          tag="tmp")
                        nc.vector.tensor_scalar(
                            tmp[:], atp[:, q * P:(q + 1) * P],
                            rss[it][:], None, OP.mult)
                        xn = xsp.tile([P, D], F32, name=f"xs{l + 1}_{it}",
                                      tag=f"xs{it}")
                        nc.vector.tensor_tensor(xn[:], tmp[:], xs[it][:],
                                                OP.add)
                        xs_new.append(xn)
                xs = xs_new

                # transpose new shard -> xTs
                xTs = xtp.tile([P, NS], F32, name=f"xTs{l + 1}", tag="xTs")
                for g in range(nit // 4):
                    pst = pe_pool.tile([P, 512], F32, name=f"xtp{l}_{g}",
                                       tag="pe")
                    for q in range(4):
                        nc.tensor.transpose(pst[:, q * P:(q + 1) * P],
                                            xs[g * 4 + q][:], ident[:])
                    nc.vector.tensor_copy(xTs[:, g * 512:(g + 1) * 512],
                                          pst[:])

                if not last:
                    # exchange shards within the pair -> full xT
                    nc.gpsimd.dma_start(ag_in[l].ap(), xTs[:])
                    nc.gpsimd.collective_compute(
                        "AllGather", OP.bypass, replica_groups=pair_groups,
                        ins=[ag_in[l].ap()], outs=[ag_out[l].ap()])
                    xT = xtp.tile([P, N], F32, name=f"xT{l + 1}", tag="xT",
                                  bufs=3)
                    nc.gpsimd.dma_start(xT[:, 0:NS], ag_out[l].ap()[0:P, :])
                    nc.gpsimd.dma_start(xT[:, NS:N],
                                        ag_out[l].ap()[P:2 * P, :])

            # ---- final linear: out = relu(x @ Wt.T + bt) ----
            for g in range(nH):
                for ch in range(nch):
                    ps = pe_pool.tile([P, 512], F32, name=f"ops{g}_{ch}",
                                      tag="pe")
                    nc.tensor.matmul(ps[:], WtTt[:, g * P:(g + 1) * P],
                                     xTs[:, ch * 512:(ch + 1) * 512],
                                     start=True, stop=True)
                    oc = ocp.tile([P, 512], F32, name=f"oc{g}_{ch}",
                                  tag="oc")
                    nc.vector.tensor_scalar(oc[:], ps[:], btpt[:, g:g + 1],
                                            0.0, OP.add, OP.max)
                    pst = pe_pool.tile([P, 512], F32, name=f"otp{g}_{ch}",
                                       tag="pe")
                    for q in range(4):
                        nc.tensor.transpose(pst[:, q * P:(q + 1) * P],
                                            oc[:, q * P:(q + 1) * P],
                                            ident[:])
                    for q in range(4):
                        k = ch * 4 + q
                        ob = smallp.tile([P, P], F32, name=f"ob{g}_{k}",
                                         tag="ob")
                        nc.vector.tensor_copy(ob[:],
                                              pst[:, q * P:(q + 1) * P])
                        nc.sync.dma_start(
                            out_ext.ap()[k * P:(k + 1) * P,
                                         g * P:(g + 1) * P],
                            ob[:])

    if legalize:
        # semantics-preserving; skip for CoreSim runs (its race detector
        # rejects post-Tile instruction insertion)
        _legalize_waits(nc)
    return nc


def make_in_maps(x, adj, Ws, bs, avs, Wt, bt, num_cores, NS):
    """Per-core input dicts. Core c -> (graph c//2, row-half c%2)."""
    B, N, D = x.shape
    H = Wt.shape[0]
    nH = H // P
    x = np.ascontiguousarray(x, np.float32)
    adj = np.ascontiguousarray(adj, np.int32)
    shared = {"WtT": np.ascontiguousarray(np.asarray(Wt, np.float32).T),
              "btp": np.ascontiguousarray(
                  np.asarray(bt, np.float32).reshape(nH, P).T)}
    for l, (W, b, a) in enumerate(zip(Ws, bs, avs)):
        shared[f"WT{l}"] = np.ascontiguousarray(np.asarray(W, np.float32).T)
        shared[f"bv{l}"] = np.ascontiguousarray(
            np.asarray(b, np.float32).reshape(D, 1))
        shared[f"av{l}"] = np.ascontiguousarray(
            np.stack([np.asarray(a, np.float32)[:D, 0],
                      np.asarray(a, np.float32)[D:, 0]], axis=1))
    in_maps = []
    for c in range(num_cores):
        b, s = c // 2, c % 2
        m = dict(shared)
        m["xT"] = np.ascontiguousarray(x[b].T)
        m["xTs"] = np.ascontiguousarray(x[b, s * NS:(s + 1) * NS].T)
        m["xs"] = np.ascontiguousarray(x[b, s * NS:(s + 1) * NS])
        m["adj_s"] = np.ascontiguousarray(adj[b, s * NS:(s + 1) * NS, :])
        in_maps.append(m)
    return in_maps


_NC_CACHE = {}


def kernel(x, adj, W0, b0, W1, b1, W2, b2, a0, a1, a2, Wt, bt):
    B, N, D = 4, 2048, 128
    H = 256
    NUM_CORES = 8
    NS = N // 2
    pair_groups = [[2 * i, 2 * i + 1] for i in range(NUM_CORES // 2)]

    key = (N, NS, D, H, NUM_CORES)
    if key not in _NC_CACHE:
        _NC_CACHE[key] = build_gat_nc(N, NS, D, H, NUM_CORES, pair_groups)
    nc = _NC_CACHE[key]

    in_maps = make_in_maps(np.asarray(x), np.asarray(adj),
                           [W0, W1, W2], [b0, b1, b2], [a0, a1, a2],
                           np.asarray(Wt), np.asarray(bt), NUM_CORES, NS)
    res = run_bass_kernel_spmd(nc, in_maps, list(range(NUM_CORES))).results
    out = np.empty((B, N, H), np.float32)
    for c in range(NUM_CORES):
        b, s = c // 2, c % 2
        out[b, s * NS:(s + 1) * NS, :] = res[c]["out_s"]
    return out


# revision 21
# speedup vs baseline: 1.0472x; 1.0472x over previous
"""GAT (3-layer graph attention + final linear) Trainium2 Bass kernel.

Problem: B=4 graphs, N=2048 atoms, D=128, H=256.
  per layer: h = relu(x @ W.T + b); e_ij = leaky_relu(f1_i + f2_j, 0.01)
  masked by adj; att = softmax_j(e); x = x + att @ h.
  final: relu(x @ Wt.T + bt).

Sharding: 8 cores; core c -> (graph b=c//2, row-half s=c%2 of the NxN
attention). Each core computes full h for its graph (cheap), attention
only for its 1024 rows. Between layers the updated x rows are exchanged
within (2b, 2b+1) pairs via AllGather.

Key tricks:
  - additive mask maskT[j,i] = 9e15*adj[i,j] - 9e15 precomputed once
    (transposed via PE, stored bf16), kept in SBUF; each layer a PE
    identity-matmul writes it into PSUM (start=True) and one K=2 outer
    product [f2;1]x[1;f1] accumulates the logits on top, so masking
    costs no separate NxN pass (and no DVE time).
  - softmax without row-max: logits bounded (~36) so exp is safe in f32;
    validated vs reference (rel err 3.6e-7).
  - exp(leaky(z)) = max(exp(z), exp(0.01*z)): two ACT Exp passes (same
    LUT set, no table reloads) + one DVE max.
  - attention aggregated transposed (attoutT = hnat[j]^T @ p, one
    stationary load per j-tile, 512-wide bf16 streams); row sums via a
    ones-column matmul; normalization happens after transposing back,
    on [NS,D] instead of an NxN pass.

Hardware wait-slot discipline (walrus limits: DMA instr = 1 sem wait,
engine instr = 2): every DMA is a first-write to its slot or its slot
was last touched by a single full-coverage engine write (gpsimd memset);
small weights are "laundered" through DVE copies so no matmul depends on
two DMA queues.
"""

import numpy as np

import concourse.bass as bass
import concourse.mybir as mybir
import concourse.tile as tile
from concourse import masks
from concourse.bass_utils import run_bass_kernel_spmd

P = 128
F32 = mybir.dt.float32
BF16 = mybir.dt.bfloat16
I32 = mybir.dt.int32
AF = mybir.ActivationFunctionType
OP = mybir.AluOpType

NEGC = 9e15


def _legalize_waits(nc, dma_limit=1, engine_limit=1):
    """Walrus can encode only 1 sem wait on a DMA instruction and ~2 on an
    engine instruction. Move excess waits onto standalone EventSemaphore
    instructions (1 wait each) inserted just before the offender on the
    same engine."""
    counter = [0]

    def split(ins):
        si = ins.sync_info
        if si is None:
            return None
        limit = dma_limit if type(ins).__name__.startswith("InstDMA") \
            else engine_limit
        waits = list(si.on_wait)
        if len(waits) <= limit:
            return None
        keep = waits[-limit:] if limit > 0 else []
        extra = waits[:-limit] if limit > 0 else waits
        evs = []
        for w in extra:
            counter[0] += 1
            evs.append(mybir.InstEventSemaphore(
                name=f"evsplit{counter[0]}", engine=ins.engine,
                sync_info=mybir.SyncInfo(on_wait=[w], on_update=[])))
        ins.sync_info = mybir.SyncInfo(on_wait=keep,
                                       on_update=list(si.on_update))
        return evs

    for f in nc.m.functions:
        for blk in f.blocks:
            new_list = []
            changed = False
            for ins in blk.instructions:
                evs = split(ins)
                if evs:
                    new_list.extend(evs)
                    changed = True
                new_list.append(ins)
            if changed:
                blk.instructions = new_list


def build_gat_nc(N, NS, D, H, num_cores, pair_groups, nlayers=3,
                 legalize=True):
    assert D == P and NS % 512 == 0 and N % 512 == 0
    nj = N // P        # j tiles
    nit = NS // P      # i tiles in shard
    nch = NS // 512    # 512-chunks in shard
    nchN = N // 512
    nH = H // P

    nc = bass.Bass("TRN2", target_bir_lowering=False, debug=False,
                   num_devices=num_cores)

    # ---- I/O ----
    xT_in = nc.dram_tensor("xT", [P, N], F32, kind="ExternalInput")
    xTs_in = nc.dram_tensor("xTs", [P, NS], F32, kind="ExternalInput")
    xs_in = nc.dram_tensor("xs", [NS, D], F32, kind="ExternalInput")
    adj_in = nc.dram_tensor("adj_s", [NS, N], I32, kind="ExternalInput")
    WT_in = [nc.dram_tensor(f"WT{l}", [D, D], F32, kind="ExternalInput")
             for l in range(nlayers)]
    bv_in = [nc.dram_tensor(f"bv{l}", [D, 1], F32, kind="ExternalInput")
             for l in range(nlayers)]
    av_in = [nc.dram_tensor(f"av{l}", [D, 2], F32, kind="ExternalInput")
             for l in range(nlayers)]
    WtT_in = nc.dram_tensor("WtT", [D, H], F32, kind="ExternalInput")
    btp_in = nc.dram_tensor("btp", [P, nH], F32, kind="ExternalInput")
    out_ext = nc.dram_tensor("out_s", [NS, H], F32, kind="ExternalOutput")

    # DRAM bounce buffers for the pair AllGather of xT shards
    ag_in = [nc.dram_tensor(f"ag_in{l}", [P, NS], F32)
             for l in range(nlayers - 1)]
    ag_out = [nc.dram_tensor(f"ag_out{l}", [2 * P, NS], F32)
              for l in range(nlayers - 1)]

    with tile.TileContext(nc) as tc:
        import contextlib
        ctx = contextlib.ExitStack()
        with ctx:
            persist = ctx.enter_context(tc.tile_pool(name="persist", bufs=1))
            rawp = ctx.enter_context(tc.tile_pool(name="rawp", bufs=2))
            convp = ctx.enter_context(tc.tile_pool(name="convp", bufs=4))
            qp = ctx.enter_context(tc.tile_pool(name="qp", bufs=2))
            xsp = ctx.enter_context(tc.tile_pool(name="xsp", bufs=2))
            xtp = ctx.enter_context(tc.tile_pool(name="xtp", bufs=2))
            smallp = ctx.enter_context(tc.tile_pool(name="smallp", bufs=4))
            ocp = ctx.enter_context(tc.tile_pool(name="ocp", bufs=2))
            pe_pool = ctx.enter_context(
                tc.tile_pool(name="pe_pool", bufs=2, space="PSUM"))
            attp = ctx.enter_context(
                tc.tile_pool(name="attp", bufs=1, space="PSUM"))
            spp = ctx.enter_context(
                tc.tile_pool(name="spp", bufs=1, space="PSUM"))

            ident = persist.tile([P, P], F32)
            masks.make_identity(nc, ident[:])
            identb = persist.tile([P, P], BF16)
            masks.make_identity(nc, identb[:])

            # ---- persistent state ----
            maskT = [persist.tile([P, NS], BF16, name=f"maskT{j}",
                                  tag=f"maskT{j}") for j in range(nj)]
            hT = persist.tile([P, N], F32)
            hsT = persist.tile([P, NS], F32)
            hnat = [persist.tile([P, D], BF16, name=f"hnat{j}",
                                 tag=f"hnat{j}") for j in range(nj)]
            onesrow = persist.tile([1, N], F32)
            nc.vector.memset(onesrow[:], 1.0)
            onescol = persist.tile([P, 1], BF16)
            nc.vector.memset(onescol[:], 1.0)
            f1row = persist.tile([1, NS], F32)
            frowA = persist.tile([2, N], F32)   # [f2 ; ones]
            frowB = persist.tile([2, NS], F32)  # [ones ; f1]
            nc.sync.dma_start(frowA[1:2, :], onesrow[:])
            nc.sync.dma_start(frowB[0:1, :], onesrow[:, 0:NS])

            # raw DMA'd weights + DVE-laundered copies (so matmuls never
            # depend on two DMA queues)
            WT_d = [persist.tile([D, D], F32, name=f"WTd{l}", tag=f"WTd{l}")
                    for l in range(nlayers)]
            bv_d = [persist.tile([D, 1], F32, name=f"bvd{l}", tag=f"bvd{l}")
                    for l in range(nlayers)]
            av_d = [persist.tile([D, 2], F32, name=f"avd{l}", tag=f"avd{l}")
                    for l in range(nlayers)]
            WtT_d = persist.tile([D, H], F32)
            btp_d = persist.tile([P, nH], F32)
            WT = [persist.tile([D, D], F32, name=f"WTl{l}", tag=f"WTl{l}")
                  for l in range(nlayers)]
            bv = [persist.tile([D, 1], F32, name=f"bvl{l}", tag=f"bvl{l}")
                  for l in range(nlayers)]
            av = [persist.tile([D, 2], F32, name=f"avl{l}", tag=f"avl{l}")
                  for l in range(nlayers)]
            WtTt = persist.tile([D, H], F32)
            btpt = persist.tile([P, nH], F32)
            for l in range(nlayers):
                nc.sync.dma_start(WT_d[l][:], WT_in[l].ap())
                nc.sync.dma_start(bv_d[l][:], bv_in[l].ap())
                nc.sync.dma_start(av_d[l][:], av_in[l].ap())
                nc.vector.tensor_copy(WT[l][:], WT_d[l][:])
                nc.vector.tensor_copy(bv[l][:], bv_d[l][:])
                nc.vector.tensor_copy(av[l][:], av_d[l][:])
            nc.sync.dma_start(WtT_d[:], WtT_in.ap())
            nc.sync.dma_start(btp_d[:], btp_in.ap())
            nc.vector.tensor_copy(WtTt[:], WtT_d[:])
            nc.vector.tensor_copy(btpt[:], btp_d[:])

            # ---- preprocessing: maskT[j][:, i] = 9e15*adj[i, j] - 9e15 ----
            # raw slots are "closed" with a full-coverage gpsimd memset so
            # the next DMA into the slot has exactly one wait.
            for itg in range(nit // 4):
                convs = []
                for q in range(4):
                    it = itg * 4 + q
                    raw = rawp.tile([P, N], I32, name=f"raw{it}", tag="raw")
                    nc.sync.dma_start(raw[:],
                                      adj_in.ap()[it * P:(it + 1) * P, :])
                    conv = convp.tile([P, N], BF16, name=f"conv{it}",
                                      tag="conv")
                    nc.vector.tensor_scalar(conv[:], raw[:], NEGC, -NEGC,
                                            OP.mult, OP.add)
                    nc.gpsimd.memset(raw[:], 0)
                    convs.append(conv)
                for j in range(nj):
                    pst = pe_pool.tile([P, 512], BF16, name=f"tp{itg}_{j}",
                                       tag="pe")
                    for q in range(4):
                        nc.tensor.transpose(pst[:, q * P:(q + 1) * P],
                                            convs[q][:, j * P:(j + 1) * P],
                                            identb[:])
                    nc.vector.tensor_copy(
                        maskT[j][:, itg * 512:(itg + 1) * 512], pst[:])

            # ---- initial x state ----
            xT = xtp.tile([P, N], F32, name="xT0", tag="xT", bufs=3)
            nc.sync.dma_start(xT[:], xT_in.ap())
            xTs = xtp.tile([P, NS], F32, name="xTs0", tag="xTs")
            nc.sync.dma_start(xTs[:], xTs_in.ap())
            xs = []
            for k in range(nit):
                t = xsp.tile([P, D], F32, name=f"xs0_{k}", tag=f"xs{k}")
                nc.sync.dma_start(t[:], xs_in.ap()[k * P:(k + 1) * P, :])
                xs.append(t)

            for l in range(nlayers):
                last = l == nlayers - 1
                # h full (transposed): hT = relu(WT.T @ xT + b)
                for ch in range(nchN):
                    ps = pe_pool.tile([P, 512], F32, name=f"hps{l}_{ch}",
                                      tag="pe")
                    nc.tensor.matmul(ps[:], WT[l][:],
                                     xT[:, ch * 512:(ch + 1) * 512],
                                     start=True, stop=True)
                    nc.vector.tensor_scalar(hT[:, ch * 512:(ch + 1) * 512],
                                            ps[:], bv[l][:], 0.0,
                                            OP.add, OP.max)
                # h shard (transposed)
                for ch in range(nch):
                    ps = pe_pool.tile([P, 512], F32, name=f"hsps{l}_{ch}",
                                      tag="pe")
                    nc.tensor.matmul(ps[:], WT[l][:],
                                     xTs[:, ch * 512:(ch + 1) * 512],
                                     start=True, stop=True)
                    nc.vector.tensor_scalar(hsT[:, ch * 512:(ch + 1) * 512],
                                            ps[:], bv[l][:], 0.0,
                                            OP.add, OP.max)
                # f2 over all atoms / f1 over shard -> partition-0 rows
                for ch in range(nchN):
                    ps = pe_pool.tile([1, 512], F32, name=f"f2ps{l}_{ch}",
                                      tag="pe")
                    nc.tensor.matmul(ps[:], av[l][:, 1:2],
                                     hT[:, ch * 512:(ch + 1) * 512],
                                     start=True, stop=True)
                    nc.vector.tensor_copy(
                        frowA[0:1, ch * 512:(ch + 1) * 512], ps[0:1, :])
                for ch in range(nch):
                    ps = pe_pool.tile([1, 512], F32, name=f"f1ps{l}_{ch}",
                                      tag="pe")
                    nc.tensor.matmul(ps[:], av[l][:, 0:1],
                                     hsT[:, ch * 512:(ch + 1) * 512],
                                     start=True, stop=True)
                    nc.vector.tensor_copy(
                        f1row[0:1, ch * 512:(ch + 1) * 512], ps[0:1, :])
                nc.sync.dma_start(frowB[1:2, :], f1row[:])

                # hext: natural-layout h tiles (transpose hT) + ones column
                for g in range(nj // 4):
                    pst = pe_pool.tile([P, 512], F32, name=f"htp{l}_{g}",
                                       tag="pe")
                    for q in range(4):
                        j = g * 4 + q
                        nc.tensor.transpose(pst[:, q * P:(q + 1) * P],
                                            hT[:, j * P:(j + 1) * P],
                                            ident[:])
                    for q in range(4):
                        j = g * 4 + q
                        nc.vector.tensor_copy(hnat[j][:],
                                              pst[:, q * P:(q + 1) * P])

                # ---- attention + aggregation (transposed accum) ----
                psAT = attp.tile([P, NS], F32, name=f"psAT{l}", tag="att")
                psS = spp.tile([1, NS], F32, name=f"psS{l}", tag="s")
                for j in range(nj):
                    pe = pe_pool.tile([P, NS], F32, name=f"pe{l}_{j}",
                                      tag="pe")
                    for ch in range(nch):
                        sl = slice(ch * 512, (ch + 1) * 512)
                        # mask preload via PE identity-matmul (bf16)
                        nc.tensor.matmul(pe[:, sl], identb[:],
                                         maskT[j][:, sl],
                                         start=True, stop=False)
                        # += f2_j x ones + ones x f1_i  (K=2)
                        nc.tensor.matmul(pe[:, sl],
                                         frowA[0:2, j * P:(j + 1) * P],
                                         frowB[0:2, sl],
                                         start=False, stop=True)
                    # exp(leaky(z)) = max(exp(z), exp(0.01 z)), in bf16
                    q1 = qp.tile([P, NS], BF16, name=f"q1_{l}_{j}", tag="q1")
                    nc.scalar.activation(q1[:], pe[:], AF.Exp)
                    q2 = qp.tile([P, NS], BF16, name=f"q2_{l}_{j}", tag="q2")
                    nc.scalar.activation(q2[:], pe[:], AF.Exp, scale=0.01)
                    p = q1
                    nc.vector.tensor_tensor(p[:], q1[:], q2[:], OP.max)
                    for ch in range(nch):
                        sl = slice(ch * 512, (ch + 1) * 512)
                        nc.tensor.matmul(psAT[:, sl], hnat[j][:], p[:, sl],
                                         start=(j == 0), stop=(j == nj - 1))
                        nc.tensor.matmul(psS[0:1, sl], onescol[:], p[:, sl],
                                         start=(j == 0), stop=(j == nj - 1))

                # normalize + residual -> new xs tiles
                aT = qp.tile([P, NS], F32, name=f"aT{l}", tag="aT")
                nc.vector.tensor_copy(aT[:], psAT[:])
                s_row = smallp.tile([1, NS], F32, name=f"srow{l}",
                                    tag="srow")
                nc.vector.tensor_copy(s_row[:], psS[:])
                # s row -> per-partition column via PE transpose
                stp = pe_pool.tile([P, nit], F32, name=f"stp{l}", tag="pe")
                for it in range(nit):
                    nc.tensor.transpose(stp[:, it:it + 1],
                                        s_row[0:1, it * P:(it + 1) * P],
                                        ident[0:1, 0:1])
                rss = []
                for it in range(nit):
                    rs = smallp.tile([P, 1], F32, name=f"rs{l}_{it}",
                                     tag="rs", bufs=8)
                    nc.vector.reciprocal(rs[:], stp[:, it:it + 1])
                    rss.append(rs)
                xs_new = []
                for g2 in range(nit // 4):
                    atp = pe_pool.tile([P, 512], F32, name=f"atp{l}_{g2}",
                                       tag="pe")
                    for q in range(4):
                        it = g2 * 4 + q
                        nc.tensor.transpose(atp[:, q * P:(q + 1) * P],
                                            aT[:, it * P:(it + 1) * P],
                                            ident[:])
                    for q in range(4):
                        it = g2 * 4 + q
                        tmp = smallp.tile([P, D], F32, name=f"tmp{l}_{it}",
                                          tag="tmp")
                        nc.vector.tensor_scalar(
                            tmp[:], atp[:, q * P:(q + 1) * P],
                            rss[it][:], None, OP.mult)
                        xn = xsp.tile([P, D], F32, name=f"xs{l + 1}_{it}",
                                      tag=f"xs{it}")
                        nc.vector.tensor_tensor(xn[:], tmp[:], xs[it][:],
                                                OP.add)
                        xs_new.append(xn)
                xs = xs_new

                # transpose new shard -> xTs
                xTs = xtp.tile([P, NS], F32, name=f"xTs{l + 1}", tag="xTs")
                for g in range(nit // 4):
                    pst = pe_pool.tile([P, 512], F32, name=f"xtp{l}_{g}",
                                       tag="pe")
                    for q in range(4):
                        nc.tensor.transpose(pst[:, q * P:(q + 1) * P],
                                            xs[g * 4 + q][:], ident[:])
                    nc.vector.tensor_copy(xTs[:, g * 512:(g + 1) * 512],
                                          pst[:])

                if not last:
                    # exchange shards within the pair -> full xT
                    nc.gpsimd.dma_start(ag_in[l].ap(), xTs[:])
                    nc.gpsimd.collective_compute(
                        "AllGather", OP.bypass, replica_groups=pair_groups,
                        ins=[ag_in[l].ap()], outs=[ag_out[l].ap()])
                    xT = xtp.tile([P, N], F32, name=f"xT{l + 1}", tag="xT",
                                  bufs=3)
                    nc.gpsimd.dma_start(xT[:, 0:NS], ag_out[l].ap()[0:P, :])
                    nc.gpsimd.dma_start(xT[:, NS:N],
                                        ag_out[l].ap()[P:2 * P, :])

            # ---- final linear: out = relu(x @ Wt.T + bt) ----
            for g in range(nH):
                for ch in range(nch):
                    ps = pe_pool.tile([P, 512], F32, name=f"ops{g}_{ch}",
                                      tag="pe")
                    nc.tensor.matmul(ps[:], WtTt[:, g * P:(g + 1) * P],
                                     xTs[:, ch * 512:(ch + 1) * 512],
                                     start=True, stop=True)
                    oc = ocp.tile([P, 512], F32, name=f"oc{g}_{ch}",
                                  tag="oc")
                    nc.vector.tensor_scalar(oc[:], ps[:], btpt[:, g:g + 1],
                                            0.0, OP.add, OP.max)
                    pst = pe_pool.tile([P, 512], F32, name=f"otp{g}_{ch}",
                                       tag="pe")
                    for q in range(4):
                        nc.tensor.transpose(pst[:, q * P:(q + 1) * P],
                                            oc[:, q * P:(q + 1) * P],
                                            ident[:])
                    for q in range(4):
                        k = ch * 4 + q
                        ob = smallp.tile([P, P], F32, name=f"ob{g}_{k}",
                                         tag="ob")
                        nc.vector.tensor_copy(ob[:],
                                              pst[:, q * P:(q + 1) * P])
                        nc.sync.dma_start(
                            out_ext.ap()[k * P:(k + 1) * P,
                                         g * P:(g + 1) * P],
                            ob[:])

    if legalize:
        # semantics-preserving; skip for CoreSim runs (its race detector
        # rejects post-Tile instruction insertion)
        _legalize_waits(nc)
    return nc


def make_in_maps(x, adj, Ws, bs, avs, Wt, bt, num_cores, NS):
    """Per-core input dicts. Core c -> (graph c//2, row-half c%2)."""
    B, N, D = x.shape
    H = Wt.shape[0]
    nH = H // P
    x = np.ascontiguousarray(x, np.float32)
    adj = np.ascontiguousarray(adj, np.int32)
    shared = {"WtT": np.ascontiguousarray(np.asarray(Wt, np.float32).T),
              "btp": np.ascontiguousarray(
                  np.asarray(bt, np.float32).reshape(nH, P).T)}
    for l, (W, b, a) in enumerate(zip(Ws, bs, avs)):
        shared[f"WT{l}"] = np.ascontiguousarray(np.asarray(W, np.float32).T)
        shared[f"bv{l}"] = np.ascontiguousarray(
            np.asarray(b, np.float32).reshape(D, 1))
        shared[f"av{l}"] = np.ascontiguousarray(
            np.stack([np.asarray(a, np.float32)[:D, 0],
                      np.asarray(a, np.float32)[D:, 0]], axis=1))
    in_maps = []
    for c in range(num_cores):
        b, s = c // 2, c % 2
        m = dict(shared)
        m["xT"] = np.ascontiguousarray(x[b].T)
        m["xTs"] = np.ascontiguousarray(x[b, s * NS:(s + 1) * NS].T)
        m["xs"] = np.ascontiguousarray(x[b, s * NS:(s + 1) * NS])
        m["adj_s"] = np.ascontiguousarray(adj[b, s * NS:(s + 1) * NS, :])
        in_maps.append(m)
    return in_maps


_NC_CACHE = {}


def kernel(x, adj, W0, b0, W1, b1, W2, b2, a0, a1, a2, Wt, bt):
    B, N, D = 4, 2048, 128
    H = 256
    NUM_CORES = 8
    NS = N // 2
    pair_groups = [[2 * i, 2 * i + 1] for i in range(NUM_CORES // 2)]

    key = (N, NS, D, H, NUM_CORES)
    if key not in _NC_CACHE:
        _NC_CACHE[key] = build_gat_nc(N, NS, D, H, NUM_CORES, pair_groups)
    nc = _NC_CACHE[key]

    in_maps = make_in_maps(np.asarray(x), np.asarray(adj),
                           [W0, W1, W2], [b0, b1, b2], [a0, a1, a2],
                           np.asarray(Wt), np.asarray(bt), NUM_CORES, NS)
    res = run_bass_kernel_spmd(nc, in_maps, list(range(NUM_CORES))).results
    out = np.empty((B, N, H), np.float32)
    for c in range(NUM_CORES):
        b, s = c // 2, c % 2
        out[b, s * NS:(s + 1) * NS, :] = res[c]["out_s"]
    return out
